# revision 1
# baseline (speedup 1.0000x reference)
"""HNetv1 Trainium2 Bass kernel.

Strategy (8 NeuronCores):
  - Every core computes the l2-normalized correlation for ALL 64 batches
    (needed as the full activation input for tensor-parallel layer 1).
  - Layer 1 (x[64,20736] @ w1[20736,5184]) is column-split 8 ways: each core
    holds a 648-column slice of w1 (bf16). 88 of the 162 k-tiles are
    prefetched into an SBUF circular buffer during the corr phase (chunked
    onto the scalar-engine HWDGE ring so they don't serialize ahead of the
    corr input loads, which use the sync ring); the remaining 74 stream
    into freed slots (paired refills) during the L1 matmul loop.
  - Layer 2 is ROW-split on the contraction dim: each core computes the
    partial h2 = h1_slice @ w2[648c:648(c+1), :] for all 64 batches, then a
    single AllToAll(bf16) exchanges batch-shards of the partials; each core
    sums its 8 received shards with one sel-matmul per 432-col chunk,
    yielding its own 8 batches. b2 is pre-divided by 8 on the host so the
    per-core bias adds sum to b2. Layers 3/4 then run on just 8 batches
    per core; the host concatenates the per-core [8,8] outputs.
  - A dummy 128-byte AllGather fires at the start of L1 so the cores resync
    there; the real AllToAll then pays ~1us of skew instead of ~11us.
  - A burst of ones-matmuls at kernel start warms the PE HAM clock gate
    (1.2 -> 2.4 GHz) while the first input DMA is in flight; the corr ssq
    matmuls consume its (rescaled) output so it cannot be scheduled late.
  - The x1-side l2 normalization is folded into the corr PSUM->SBUF drain
    as a per-partition scale (0-stride broadcast over k), so only x2 pays
    the full-size normalize multiply.
  - All matmul operands are bf16 (host-converted); accumulation is fp32 PSUM.

Layouts (validated against the reference in numpy):
  x1r/x2r: [C=128, N=64, HW=144] bf16 (host-transposed).
  corr^T for batch n is produced as psum [ij, k]; X_a[ij 0:128, k, n] holds the
  main part, the ij residue (16 rows) is staged in X_btmp[16, k, n] and
  regrouped by an SBUF->SBUF DMA into X_b[16*bi+r, bo, n] with k = 8*bo+bi.
  w1m (per core): [162, 128, 648] where tiles 0..143 are w1[k*144:(k*144+128)]
  and tiles 144+bo hold the gathered residue rows — so L1 is 162 plain
  [128,64]x[128,648] accumulating matmuls.
"""

import os
import numpy as np
import ml_dtypes

N, C, S = 64, 128, 12
HW = S * S            # 144
RIN = S ** 4          # 20736
NCORES = 8
COLS1 = 5184 // NCORES   # 648
NB = N // NCORES         # 8 batches per core after ReduceScatter
G = 4                    # batches per corr group
NGRP = N // G            # 16
PRE = 88                 # w1 k-tiles prefetched to SBUF (of 162)
KT1 = 162                # total w1 k-tiles
D2 = 1296
D3 = 324
PADK2 = 768              # 648 padded to 6*128
PADK3 = 1408             # 1296 padded to 11*128
PADK4 = 384              # 324 padded to 3*128

_CACHE = {}

LAST_RESULT = None  # BassKernelResults from the most recent run (for test.py)


def _bf16(a):
    return np.asarray(a, dtype=np.float32).astype(ml_dtypes.bfloat16)


def _build_nc(trace_enabled=False):
    import concourse.bacc as bacc
    import concourse.tile as tile
    import concourse.mybir as mybir
    from concourse.masks import make_identity

    dt = mybir.dt
    AF = mybir.ActivationFunctionType
    ALU = mybir.AluOpType

    nc = bacc.Bacc("TRN2", target_bir_lowering=False, debug=False,
                   num_devices=NCORES)

    x1r_d = nc.dram_tensor("x1r", [C, N, HW], dt.bfloat16, kind="ExternalInput")
    x2r_d = nc.dram_tensor("x2r", [C, N, HW], dt.bfloat16, kind="ExternalInput")
    w1m_d = nc.dram_tensor("w1m", [KT1, 128, COLS1], dt.bfloat16, kind="ExternalInput")
    b1s_d = nc.dram_tensor("b1s", [1, COLS1], dt.bfloat16, kind="ExternalInput")
    w2r_d = nc.dram_tensor("w2r", [PADK2, D2], dt.bfloat16, kind="ExternalInput")
    b2f_d = nc.dram_tensor("b2f", [1, D2], dt.bfloat16, kind="ExternalInput")
    w3f_d = nc.dram_tensor("w3f", [PADK3, D3], dt.bfloat16, kind="ExternalInput")
    b3_d = nc.dram_tensor("b3r", [1, D3], dt.bfloat16, kind="ExternalInput")
    w4p_d = nc.dram_tensor("w4p", [PADK4, 8], dt.bfloat16, kind="ExternalInput")
    b4_d = nc.dram_tensor("b4r", [1, 8], dt.bfloat16, kind="ExternalInput")
    out_d = nc.dram_tensor("out", [NB, 8], dt.float32, kind="ExternalOutput")

    rg = [list(range(NCORES))]

    with tile.TileContext(nc) as tc:
        with tc.tile_pool(name="persist", bufs=1) as persist, \
             tc.tile_pool(name="dramp", bufs=1, space="DRAM") as dramp:
            # internal DRAM for the AllToAll exchange of L2 partials
            a2a_in = dramp.tile([N, D2], dt.bfloat16)
            a2a_out = dramp.tile([N, D2], dt.bfloat16)
            # dummy collective used purely as a cross-core resync point
            sync_in = dramp.tile([1, 64], dt.bfloat16)
            sync_out = dramp.tile([NCORES, 64], dt.bfloat16, addr_space="Shared")
            ones128 = persist.tile([128, 128], dt.bfloat16)
            nc.vector.memset(ones128[:], 1.0)
            onesrow = persist.tile([1, N], dt.bfloat16)
            nc.vector.memset(onesrow[:], 1.0)
            ident = persist.tile([128, 128], dt.bfloat16)
            make_identity(nc, ident[:])
            nc.sync.dma_start(sync_in[:, :], onesrow[:])
            # selT[8r+b, b] = 1: sums the 8 AllToAll shards via one matmul
            selT = persist.tile([64, 8], dt.bfloat16)
            for r in range(NCORES):
                nc.sync.dma_start(selT[8 * r:8 * r + 8, :], ident[0:8, 0:8])

            # PE warm-up: ~5us of back-to-back matmuls flips the HAM clock
            # gate to 8/8 while the first input DMA is in flight. The corr
            # ssq matmuls consume ones128b (= wu/128), which forces the
            # scheduler to actually run these first.
            ones128b = persist.tile([128, 128], dt.bfloat16)
            with tc.tile_pool(name="wub", bufs=1) as wub, \
                 tc.tile_pool(name="pwu", bufs=1, space="PSUM") as pwu:
                ones512 = wub.tile([128, 512], dt.bfloat16, tag="o512")
                nc.vector.memset(ones512[:], 1.0)
                wu = pwu.tile([128, 512], dt.float32, tag="wu")
                for _ in range(12):
                    nc.tensor.matmul(wu[:], ones128[:], ones512[:],
                                     start=True, stop=True)
                nc.vector.tensor_scalar_mul(ones128b[:], wu[:, 0:128], 1.0 / 128.0)

            X_a = persist.tile([128, HW, N], dt.bfloat16)   # [ij, k, n]
            X_b = persist.tile([128, 18, N], dt.bfloat16)   # [16*bi+r, bo, n]

            # w1 circular buffer: slots 0..PRE-1 prefilled with k-tiles
            # 0..PRE-1 during the corr phase (chunked dma_starts spread
            # through the corr loop on the scalar HWDGE ring so they never
            # block corr compute); the L1 loop refills slot t%PRE with tile
            # t+PRE as soon as tile t is consumed.
            w1pre = persist.tile([128, PRE, COLS1], dt.bfloat16)
            CH = PRE // 8

            def _prefetch_chunk(ch):
                nc.scalar.dma_start(
                    w1pre[:, CH * ch:CH * (ch + 1), :],
                    w1m_d[CH * ch:CH * (ch + 1), :, :].rearrange("t p c -> p t c"))

            _prefetch_chunk(0)

            # ---------------- corr phase ----------------
            # corr[n, ij, k] = (x1[n,:,ij]/|x1[n,:,ij]|) . (x2[n,:,k]/|x2[n,:,k]|)
            # x2 is normalized on the input side (needs the per-k norm
            # broadcast over channel partitions); x1's normalization is
            # folded into the PSUM->SBUF copy as a per-partition (=per-ij)
            # scale, so the big per-element x1 multiply disappears.
            with tc.tile_pool(name="cbt", bufs=1) as cbt, \
                 tc.tile_pool(name="cx", bufs=3) as cx, \
                 tc.tile_pool(name="csq", bufs=2) as csq, \
                 tc.tile_pool(name="cs", bufs=2) as cs, \
                 tc.tile_pool(name="cxs", bufs=2) as cxs, \
                 tc.tile_pool(name="pssq", bufs=1, space="PSUM") as pssq, \
                 tc.tile_pool(name="pca", bufs=2, space="PSUM") as pca, \
                 tc.tile_pool(name="pcb", bufs=1, space="PSUM") as pcb:
                X_btmp = cbt.tile([16, HW, N], dt.bfloat16)   # [r, k, n]
                for g in range(NGRP):
                    n0 = G * g
                    x1t = cx.tile([C, G, HW], dt.bfloat16, tag="x1t")
                    nc.sync.dma_start(x1t[:], x1r_d[:, n0:n0 + G, :])
                    x2t = cx.tile([C, G, HW], dt.bfloat16, tag="x2t")
                    nc.sync.dma_start(x2t[:], x2r_d[:, n0:n0 + G, :])

                    sq1 = csq.tile([C, G, HW], dt.bfloat16, tag="sq1")
                    nc.gpsimd.tensor_tensor(sq1[:], x1t[:], x1t[:], ALU.mult)
                    sq2 = csq.tile([C, G, HW], dt.bfloat16, tag="sq2")
                    nc.scalar.activation(sq2[:], x2t[:], AF.Square)

                    # x2 col norms broadcast to all partitions via ones-matmul;
                    # x1 row norms ([ij, b] transposed layout) are packed into
                    # spare columns of the same PSUM tile: cols 400:404 main
                    # (ij 0:128), cols 404:408 the 16-row ij residue
                    ssq = pssq.tile([128, 2, 512], dt.float32, tag="ssq")
                    for h in range(2):
                        nc.tensor.matmul(ssq[:, h, 0:2 * HW], ones128b[:],
                                         sq2[:, 2 * h:2 * h + 2, :],
                                         start=True, stop=True)
                    for b in range(G):
                        nc.tensor.matmul(ssq[:, 0, 400 + b:401 + b],
                                         sq1[:, b, 0:128],
                                         ones128[:, 0:1], start=True, stop=True)
                        nc.tensor.matmul(ssq[0:16, 0, 404 + b:405 + b],
                                         sq1[:, b, 128:HW],
                                         ones128[:, 0:1], start=True, stop=True)
                    s2 = cs.tile([128, G, HW], dt.float32, tag="s2")
                    nc.scalar.activation(
                        s2[:].rearrange("p (a b) k -> p a (b k)", b=2),
                        ssq[:, :, 0:2 * HW], AF.Sqrt)
                    r2 = cs.tile([128, G, HW], dt.float32, tag="r2")
                    nc.vector.reciprocal_approx_fast(r2[:], s2[:])
                    x2s = cxs.tile([C, G, HW], dt.bfloat16, tag="x2s")
                    nc.vector.tensor_tensor(x2s[:], x2t[:], r2[:], ALU.mult)

                    sn = cs.tile([128, 8], dt.float32, tag="sn")
                    nc.scalar.activation(sn[0:16, 4:8], ssq[0:16, 0, 404:408],
                                         AF.Sqrt)
                    nc.scalar.activation(sn[:, 0:4], ssq[:, 0, 400:404], AF.Sqrt)
                    rn = cs.tile([128, 8], dt.float32, tag="rn")
                    nc.vector.reciprocal_approx_fast(rn[0:16, 4:8], sn[0:16, 4:8])
                    nc.vector.reciprocal_approx_fast(rn[:, 0:4], sn[:, 0:4])

                    ca = pca.tile([128, 2, 512], dt.float32, tag="ca")
                    cb = pcb.tile([16, 2, 512], dt.float32, tag="cb")
                    for b in range(G):
                        nc.tensor.matmul(ca[:, b // 2, HW * (b % 2):HW * (b % 2) + HW],
                                         x1t[:, b, 0:128], x2s[:, b, :],
                                         start=True, stop=True)
                        nc.tensor.matmul(cb[:, b // 2, HW * (b % 2):HW * (b % 2) + HW],
                                         x1t[:, b, 128:HW], x2s[:, b, :],
                                         start=True, stop=True)
                    # psum -> X with the x1-norm scale fused in: one fused
                    # 4-batch op, scale broadcast over k via a 0-stride dim.
                    # Residue first so the single-buffered cb frees earliest.
                    rnb = (rn[0:16, 4:8].rearrange("p (a b) -> p a b", b=2)
                           .unsqueeze(1).broadcast_to([16, HW, 2, 2]))
                    nc.vector.tensor_tensor(
                        X_btmp[:, :, n0:n0 + G].rearrange("r k (a b) -> r k a b", b=2),
                        cb[:, :, 0:2 * HW].rearrange("r a (b k) -> r k a b", b=2),
                        rnb, ALU.mult)
                    rna = (rn[:, 0:4].rearrange("p (a b) -> p a b", b=2)
                           .unsqueeze(1).broadcast_to([128, HW, 2, 2]))
                    nc.vector.tensor_tensor(
                        X_a[:, :, n0:n0 + G].rearrange("p k (a b) -> p k a b", b=2),
                        ca[:, :, 0:2 * HW].rearrange("p a (b k) -> p k a b", b=2),
                        rna, ALU.mult)
                    if g % 2 == 1 and g // 2 + 1 < 8:
                        _prefetch_chunk(g // 2 + 1)

                # regroup residue: X_b[16*bi+r, bo, n] = X_btmp[r, 8*bo+bi, n]
                xbt = X_btmp[:].rearrange("r (bo bi) n -> r bo bi n", bi=8)
                for bi in range(8):
                    nc.sync.dma_start(X_b[16 * bi:16 * bi + 16, 0:18, :],
                                      xbt[:, :, bi, :])

            # resync the cores here (no one reads sync_out); the real
            # ReduceScatter below then starts without the accumulated skew
            nc.gpsimd.collective_compute(
                "AllGather", mybir.AluOpType.bypass, replica_groups=rg,
                ins=[sync_in[:]], outs=[sync_out[:]])

            # weights for L2..L4 — loaded during L1
            w2sb = persist.tile([128, 6, D2], dt.bfloat16)
            nc.sync.dma_start(w2sb[:], w2r_d[:].rearrange("(t p) c -> p t c", p=128))
            b2row = persist.tile([1, D2], dt.bfloat16)
            nc.sync.dma_start(b2row[:], b2f_d[:, :])
            w3sb = persist.tile([128, 11, D3], dt.bfloat16)
            nc.sync.dma_start(w3sb[:], w3f_d[:].rearrange("(t p) c -> p t c", p=128))
            b3row = persist.tile([1, D3], dt.bfloat16)
            nc.sync.dma_start(b3row[:], b3_d[:, :])
            w4sb = persist.tile([128, 3, 8], dt.bfloat16)
            nc.sync.dma_start(w4sb[:], w4p_d[:].rearrange("(t p) c -> p t c", p=128))
            b4row = persist.tile([1, 8], dt.bfloat16)
            nc.sync.dma_start(b4row[:], b4_d[:, :])

            # ---------------- L1: x @ w1 slice ----------------
            h1sb = persist.tile([64, COLS1], dt.bfloat16)
            with tc.tile_pool(name="bias", bufs=1) as biasp, \
                 tc.tile_pool(name="ph1", bufs=1, space="PSUM") as ph1:
                b1row = biasp.tile([1, COLS1], dt.bfloat16, tag="b1")
                nc.sync.dma_start(b1row[:], b1s_d[:, :])
                h1ps = [ph1.tile([64, 324], dt.float32, tag=f"h1ps{h}", name=f"h1ps{h}")
                        for h in range(2)]
                for t in range(KT1):
                    slot = t % PRE
                    lhsT = X_a[:, t, :] if t < 144 else X_b[:, t - 144, :]
                    for h in range(2):
                        nc.tensor.matmul(h1ps[h][:], lhsT,
                                         w1pre[:, slot, 324 * h:324 * h + 324],
                                         start=(t == 0), stop=False)
                    if t % 2 == 1 and t + PRE < KT1:
                        # paired refill: tiles (t-1+PRE, t+PRE) into slots
                        # (t-1, t) — bigger DMAs drain the ring faster
                        nc.scalar.dma_start(
                            w1pre[:, slot - 1:slot + 1, :],
                            w1m_d[t + PRE - 1:t + PRE + 1, :, :].rearrange("t p c -> p t c"))
                for h in range(2):
                    nc.tensor.matmul(h1ps[h][:], onesrow[:],
                                     b1row[:, 324 * h:324 * h + 324],
                                     start=False, stop=True)
                for h in range(2):
                    nc.scalar.activation(h1sb[:, 324 * h:324 * h + 324],
                                         h1ps[h][:], AF.Relu)

            # transpose h1 -> [648, 64] (zero-padded to 768 rows)
            h1T = persist.tile([128, 6, N], dt.bfloat16)
            nc.vector.memset(h1T[:], 0.0)
            with tc.tile_pool(name="ptp", bufs=2, space="PSUM") as ptp:
                for t in range(6):
                    w = 128 if t < 5 else COLS1 - 5 * 128  # 8
                    tp = ptp.tile([128, 64], dt.bfloat16, tag="tp")
                    nc.tensor.transpose(tp[0:w, :], h1sb[:, 128 * t:128 * t + w],
                                        ident[0:64, 0:64])
                    nc.vector.tensor_copy(h1T[0:w, t, :], tp[0:w, :])

            # ---------------- L2 partial + ReduceScatter ----------------
            with tc.tile_pool(name="l2", bufs=1) as l2p, \
                 tc.tile_pool(name="ph2", bufs=1, space="PSUM") as ph2:
                h2ps = [ph2.tile([64, 432], dt.float32, tag=f"h2ps{h}", name=f"h2ps{h}")
                        for h in range(3)]
                for t in range(6):
                    for h in range(3):
                        nc.tensor.matmul(h2ps[h][:], h1T[:, t, :],
                                         w2sb[:, t, 432 * h:432 * h + 432],
                                         start=(t == 0), stop=False)
                for h in range(3):
                    nc.tensor.matmul(h2ps[h][:], onesrow[:],
                                     b2row[:, 432 * h:432 * h + 432],
                                     start=False, stop=True)
                h2bf = l2p.tile([64, D2], dt.bfloat16, tag="h2bf")
                for h in range(3):
                    nc.vector.tensor_copy(h2bf[:, 432 * h:432 * h + 432],
                                          h2ps[h][:])
                nc.sync.dma_start(a2a_in[:, :], h2bf[:])
                # rows 8j:8j+8 (this core's partial for core j's batches) go
                # to core j; we get back the 8 cores' partials for OUR 8
                # batches, summed locally with one sel-matmul per chunk.
                nc.gpsimd.collective_compute(
                    "AllToAll", mybir.AluOpType.bypass, replica_groups=rg,
                    ins=[a2a_in[:]], outs=[a2a_out[:]])

            # ---------------- L3 on this core's 8 batches ----------------
            h2T = persist.tile([128, 11, NB], dt.bfloat16)
            nc.vector.memset(h2T[:], 0.0)
            h3sb = persist.tile([NB, D3], dt.bfloat16)
            with tc.tile_pool(name="l3", bufs=1) as l3p, \
                 tc.tile_pool(name="ptp2", bufs=2, space="PSUM") as ptp2, \
                 tc.tile_pool(name="ph3s", bufs=1, space="PSUM") as ph3s, \
                 tc.tile_pool(name="ph3", bufs=1, space="PSUM") as ph3:
                a2sb = l3p.tile([N, D2], dt.bfloat16, tag="a2sb")
                nc.sync.dma_start(a2sb[:], a2a_out[:, :])
                h2r = l3p.tile([NB, D2], dt.bfloat16, tag="h2r")
                for h in range(3):
                    hp = ph3s.tile([NB, 432], dt.float32, tag=f"h2s{h}",
                                   name=f"h2s{h}")
                    nc.tensor.matmul(hp[:], selT[:],
                                     a2sb[:, 432 * h:432 * h + 432],
                                     start=True, stop=True)
                    nc.scalar.activation(h2r[:, 432 * h:432 * h + 432],
                                         hp[:], AF.Relu)
                for t in range(11):
                    w = 128 if t < 10 else D2 - 10 * 128  # 16
                    tp = ptp2.tile([128, NB], dt.bfloat16, tag="tp2")
                    nc.tensor.transpose(tp[0:w, :], h2r[:, 128 * t:128 * t + w],
                                        ident[0:NB, 0:NB])
                    nc.vector.tensor_copy(h2T[0:w, t, :], tp[0:w, :])
                h3ps = ph3.tile([NB, D3], dt.float32, tag="h3ps")
                for t in range(11):
                    nc.tensor.matmul(h3ps[:], h2T[:, t, :], w3sb[:, t, :],
                                     start=(t == 0), stop=False)
                nc.tensor.matmul(h3ps[:], onesrow[:, 0:NB], b3row[:],
                                 start=False, stop=True)
                nc.scalar.activation(h3sb[:], h3ps[:], AF.Tanh)

            # ---------------- L4 ----------------
            h3T = persist.tile([128, 3, NB], dt.bfloat16)
            nc.vector.memset(h3T[:], 0.0)
            with tc.tile_pool(name="ptp3", bufs=2, space="PSUM") as ptp3, \
                 tc.tile_pool(name="l4", bufs=1) as l4p, \
                 tc.tile_pool(name="ph4", bufs=1, space="PSUM") as ph4:
                for t in range(3):
                    w = 128 if t < 2 else D3 - 256  # 68
                    tp = ptp3.tile([128, NB], dt.bfloat16, tag="tp3")
                    nc.tensor.transpose(tp[0:w, :], h3sb[:, 128 * t:128 * t + w],
                                        ident[0:NB, 0:NB])
                    nc.vector.tensor_copy(h3T[0:w, t, :], tp[0:w, :])
                outps = ph4.tile([NB, 8], dt.float32, tag="outps")
                for t in range(3):
                    nc.tensor.matmul(outps[:], h3T[:, t, :], w4sb[:, t, :],
                                     start=(t == 0), stop=False)
                nc.tensor.matmul(outps[:], onesrow[:, 0:NB], b4row[:],
                                 start=False, stop=True)
                outsb = l4p.tile([NB, 8], dt.float32, tag="outsb")
                nc.vector.tensor_copy(outsb[:], outps[:])
                nc.sync.dma_start(out_d[:, :], outsb[:])

    nc.compile()
    return nc


def _prep_inputs(x1, x2, w1, b1, w2, b2, w3, b3, w4, b4):
    """Host-side shard/permute/cast. Returns per-core input maps."""
    x1f = np.asarray(x1, np.float32).reshape(N, C, HW)
    x2f = np.asarray(x2, np.float32).reshape(N, C, HW)
    x1r = _bf16(np.ascontiguousarray(x1f.transpose(1, 0, 2)))
    x2r = _bf16(np.ascontiguousarray(x2f.transpose(1, 0, 2)))
    w1 = np.asarray(w1, np.float32)
    w2 = np.asarray(w2, np.float32)
    w3 = np.asarray(w3, np.float32)
    w4 = np.asarray(w4, np.float32)

    # w3 padded to 11 k-tiles; w4 to 3
    w3pad = np.zeros((PADK3, D3), np.float32)
    w3pad[:D2] = w3
    w4pad = np.zeros((PADK4, 8), np.float32)
    w4pad[:D3] = w4
    w3b = _bf16(w3pad)
    w4b = _bf16(w4pad)
    b2f = _bf16(np.asarray(b2, np.float32) / NCORES).reshape(1, D2)
    b3r = _bf16(b3).reshape(1, D3)
    b4r = _bf16(b4).reshape(1, 8)

    in_maps = []
    for core in range(NCORES):
        w1c = w1[:, COLS1 * core:COLS1 * (core + 1)].reshape(HW, HW, COLS1)
        main = w1c[:, 0:128, :]
        res = w1c[:, 128:HW, :].reshape(18, 8, 16, COLS1).reshape(18, 128, COLS1)
        w1m = _bf16(np.ascontiguousarray(
            np.concatenate([main, res], axis=0)))
        w2pad = np.zeros((PADK2, D2), np.float32)
        w2pad[:COLS1] = w2[COLS1 * core:COLS1 * (core + 1)]
        in_maps.append({
            "x1r": x1r, "x2r": x2r,
            "w1m": w1m,
            "b1s": _bf16(b1[COLS1 * core:COLS1 * (core + 1)]).reshape(1, COLS1),
            "w2r": _bf16(w2pad),
            "b2f": b2f,
            "w3f": w3b,
            "b3r": b3r,
            "w4p": w4b,
            "b4r": b4r,
        })
    return in_maps


def kernel(x1, x2, w1, b1, w2, b2, w3, b3, w4, b4):
    global LAST_RESULT
    from concourse.bass_utils import run_bass_kernel_spmd

    if "nc" not in _CACHE:
        _CACHE["nc"] = _build_nc()
    nc = _CACHE["nc"]

    in_maps = _prep_inputs(x1, x2, w1, b1, w2, b2, w3, b3, w4, b4)
    trace = bool(int(os.environ.get("HNET_TRACE", "0")))
    res = run_bass_kernel_spmd(nc, in_maps, core_ids=list(range(NCORES)),
                               trace=trace)
    LAST_RESULT = res
    H = np.concatenate(
        [np.asarray(res.results[c]["out"], np.float32) for c in range(NCORES)],
        axis=0)
    ones = np.ones((N, 1), np.float32)
    return np.concatenate([H, ones], axis=1).reshape(N, 3, 3)



# revision 13
# speedup vs baseline: 1.1436x; 1.1436x over previous
"""HNetv1 Trainium2 Bass kernel — v2 (fp8 everywhere on the hot path).

Strategy (8 NeuronCores):
  - Inputs x1/x2 land as fp8e4 (no scale; randn fits e4m3). Every core
    computes the full correlation for all 64 batches (needed as the
    activation input for the tensor-parallel L1).
  - Correlation pipeline per group of G=8 batches:
      gpsimd: sq1 = x1*x1 (bf16)     scalar: sq2 = square(x2) (bf16)
      PE: col-tiled ones-matmuls reduce sum_c(sq) into a compact 4-strip
          psum layout [32q+p, f]; stationary consts 4.0/16.0 pre-fold the
          per-side scale so sqrt(recip(ssq)) directly yields 16/32*rsqrt
          and 8/32*rsqrt (the 1/32 is the row-tiled broadcast redundancy).
      DVE: reciprocal_approx_fast (compact);  scalar: sqrt -> bf16.
      PE: row-tiled ones-matmuls broadcast r to all 128 partitions (x32).
      DVE: x1n = x1*r1b, x2n = x2*r2b (bf16; scale 16 resp 8 folded in so
          corr psum = 128*corr_normalized, ideal fp8 range).
      PE: per batch, corr main [c,128ij]x[c,144k] -> ca[ij,(b,k)]; the
          16-row ij-residue is computed OPERAND-SWAPPED out[k,(b,ijr)]
          (plus a 16x16 corner) so drains stay 128-lane-parallel.
      drains: scalar Copy ca -> X_a fp8 [ij,k,n]; DVE copies for the
          swapped-residue [k,ijr,n] and corner.
  - L1 (x[64,20736] @ w1-slice[20736,648]) is column-split 8 ways and runs
    as 81 Double-FP8 (DoubleRow) pair-tile matmuls: lhsT = X pair
    [128,2,64], rhs = w1 pair [128,2,648] (fp8, host-scaled x1024, pair
    stride padded to 656 B for the %16 ISA rule). w1 streams from HBM in 9
    contiguous ~1.5 MB chunks issued up front on the scalar HWDGE ring.
  - L2 is row-split: h1 is transposed to fp8 pairs [128,6,64] (drain scale
    2^-9) and multiplied with fp8 w2 (x1024) in 9 DoubleRow matmuls; the
    2^-18 descale happens in the psum->bf16 drain. One AllToAll exchanges
    batch-shards of the [64,1296] partials; each core sums its 8 shards
    with sel-matmuls (b2 pre-divided by 8 on host, added at 2^18 scale).
  - L3/L4 run on 8 batches/core in bf16 (baseline structure). The host
    concatenates the per-core [8,8] outputs.
  - A dummy AllGather fires at kernel start to absorb launch skew on the
    CC queue; a PE warm-up burst flips the HAM clock gate early (the ssq
    stationary consts are derived from its output to force scheduling).

Scale bookkeeping (all powers of 2, folded on host):
  X_fp8 = 128*corr_n;  w1_fp8 = 1024*w1  => psum = 2^17 * z1
  h1_fp8 = relu(psum)*2^-9 = 2^8 * h1;   w2_fp8 = 1024*w2 => psum = 2^18*z2
  h2 = psum*2^-18 + b2/8 (bias pre-scaled 2^18 into the ones-matmul row).
"""

import os
import numpy as np
import ml_dtypes

N, C, S = 64, 128, 12
HW = S * S            # 144
RIN = S ** 4          # 20736
NCORES = 8
COLS1 = 5184 // NCORES   # 648
PADC = 656               # pair stride (bytes, fp8) — 16-aligned
NPAIR = 81               # 162 k-tiles as DoubleRow pairs
G = 8                    # batches per corr group
NGRP = N // G            # 8
NB = N // NCORES         # 8
D2 = 1296
D3 = 324
PADK3 = 1408             # 1296 padded to 11*128
PADK4 = 384              # 324 padded to 3*128

_CACHE = {}
LAST_RESULT = None


def _bf16(a):
    return np.asarray(a, dtype=np.float32).astype(ml_dtypes.bfloat16)


def _fp8(a):
    return np.clip(np.asarray(a, dtype=np.float32), -240.0, 240.0).astype(
        ml_dtypes.float8_e4m3)


def _build_nc(trace_enabled=False):
    import concourse.bacc as bacc
    import concourse.tile as tile
    import concourse.mybir as mybir

    from concourse.masks import make_identity

    dt = mybir.dt
    AF = mybir.ActivationFunctionType
    ALU = mybir.AluOpType
    DR = mybir.MatmulPerfMode.DoubleRow

    nc = bacc.Bacc("TRN2", target_bir_lowering=False, debug=False,
                   num_devices=NCORES)

    x1q_d = nc.dram_tensor("x1q", [C, N, HW], dt.float8e4, kind="ExternalInput")
    x2q_d = nc.dram_tensor("x2q", [C, N, HW], dt.float8e4, kind="ExternalInput")
    w1m_d = nc.dram_tensor("w1m", [128, NPAIR * 2 * PADC], dt.float8e4,
                           kind="ExternalInput")
    b1s_d = nc.dram_tensor("b1s", [1, COLS1], dt.bfloat16, kind="ExternalInput")
    w2m_d = nc.dram_tensor("w2m", [128, 6 * D2], dt.float8e4,
                           kind="ExternalInput")
    b2f_d = nc.dram_tensor("b2f", [1, D2], dt.bfloat16, kind="ExternalInput")
    w3f_d = nc.dram_tensor("w3f", [PADK3, D3], dt.bfloat16, kind="ExternalInput")
    b3_d = nc.dram_tensor("b3r", [1, D3], dt.bfloat16, kind="ExternalInput")
    w4p_d = nc.dram_tensor("w4p", [PADK4, 8], dt.bfloat16, kind="ExternalInput")
    b4_d = nc.dram_tensor("b4r", [1, 8], dt.bfloat16, kind="ExternalInput")
    out_d = nc.dram_tensor("out", [NB, 8], dt.float32, kind="ExternalOutput")
    dbg = os.environ.get("HNET_DEBUG", "0") == "1"
    if dbg:
        dXa_d = nc.dram_tensor("dXa", [128, HW * N], dt.float8e4,
                               kind="ExternalOutput")
        dXsw_d = nc.dram_tensor("dXsw", [128, 16 * N], dt.float8e4,
                                kind="ExternalOutput")
        dXcp_d = nc.dram_tensor("dXcp", [128, 2 * N], dt.float8e4,
                                kind="ExternalOutput")
        drcb_d = nc.dram_tensor("drcb", [128, 576], dt.bfloat16,
                                kind="ExternalOutput")
        dh1_d = nc.dram_tensor("dh1", [64, COLS1], dt.bfloat16,
                               kind="ExternalOutput")
        dh2_d = nc.dram_tensor("dh2", [64, D2], dt.bfloat16,
                               kind="ExternalOutput")

    rg = [list(range(NCORES))]

    with tile.TileContext(nc) as tc:
        with tc.tile_pool(name="persist", bufs=1) as persist, \
             tc.tile_pool(name="dramp", bufs=1, space="DRAM") as dramp:
            a2a_in = dramp.tile([N, D2], dt.bfloat16)
            a2a_out = dramp.tile([N, D2], dt.bfloat16)
            sync_in = dramp.tile([1, 64], dt.bfloat16)
            sync_out = dramp.tile([NCORES, 64], dt.bfloat16, addr_space="Shared")

            # all corr input loads first in the sync-ring FIFO
            x1ts, x2ts = [], []
            for g in range(NGRP):
                n0 = G * g
                x1t = persist.tile([C, G, HW], dt.float8e4, tag=f"x1t{g}")
                nc.sync.dma_start(x1t[:], x1q_d[:, n0:n0 + G, :])
                x1ts.append(x1t)
                x2t = persist.tile([C, G, HW], dt.float8e4, tag=f"x2t{g}")
                nc.sync.dma_start(x2t[:], x2q_d[:, n0:n0 + G, :])
                x2ts.append(x2t)

            ones128 = persist.tile([128, 128], dt.bfloat16)
            nc.vector.memset(ones128[:], 1.0)
            onesrow = persist.tile([1, N], dt.bfloat16)
            nc.vector.memset(onesrow[:], 1.0)
            ident = persist.tile([128, 128], dt.bfloat16)
            make_identity(nc, ident[:])
            nc.sync.dma_start(sync_in[:, :], onesrow[:, 0:64])
            selT = persist.tile([64, 8], dt.bfloat16)
            for r in range(NCORES):
                nc.sync.dma_start(selT[8 * r:8 * r + 8, :], ident[0:8, 0:8])

            # launch-skew absorber on the CC queue (nothing reads sync_out)
            nc.gpsimd.collective_compute(
                "AllGather", mybir.AluOpType.bypass, replica_groups=rg,
                ins=[sync_in[:]], outs=[sync_out[:]])

            # big streaming weights: w1 fp8 in 9 contiguous chunks, then w2
            w1sb = persist.tile([128, NPAIR * 2 * PADC], dt.float8e4)
            CHB = 9 * 2 * PADC     # 9 pairs per chunk
            for ch in range(9):
                nc.scalar.dma_start(w1sb[:, CHB * ch:CHB * (ch + 1)],
                                    w1m_d[:, CHB * ch:CHB * (ch + 1)])
            w2sb = persist.tile([128, 6 * D2], dt.float8e4)
            nc.scalar.dma_start(w2sb[:], w2m_d[:, :])

            # PE warm-up (~5us) to flip the HAM clock gate; the corr ssq
            # stationary consts are derived from its output so it schedules
            # first.
            c4 = persist.tile([128, 32], dt.bfloat16)
            c16 = persist.tile([128, 32], dt.bfloat16)
            with tc.tile_pool(name="wub", bufs=1) as wub, \
                 tc.tile_pool(name="pwu", bufs=1, space="PSUM") as pwu:
                ones512 = wub.tile([128, 512], dt.bfloat16, tag="o512")
                nc.vector.memset(ones512[:], 1.0)
                wu = pwu.tile([128, 512], dt.float32, tag="wu")
                for _ in range(12):
                    nc.tensor.matmul(wu[:], ones128[:], ones512[:],
                                     start=True, stop=True)
                nc.vector.tensor_scalar_mul(c4[:], wu[:, 0:32], 1.0 / 32.0)
                nc.vector.tensor_scalar_mul(c16[:], wu[:, 32:64], 1.0 / 8.0)

            # merged X: kappa 0:144 = main k-tiles [ij, n, k]; 144:160 =
            # swres tiles [k, n, ijr]. DoubleRow pairs at kappa distance 16.
            XM = persist.tile([128, N, 160], dt.float8e4)
            X_co = persist.tile([16, N, 16], dt.float8e4)     # [kr, n, ijrs]
            X_cp = persist.tile([128, 2, N], dt.float8e4)     # corner pair

            # ---------------- corr phase ----------------
            with tc.tile_pool(name="cx", bufs=1) as cx, \
                 tc.tile_pool(name="csq", bufs=2) as csq, \
                 tc.tile_pool(name="crr", bufs=2) as crr, \
                 tc.tile_pool(name="cxn", bufs=2) as cxn, \
                 tc.tile_pool(name="pnorm", bufs=1, space="PSUM") as pnorm, \
                 tc.tile_pool(name="pca", bufs=1, space="PSUM") as pca, \
                 tc.tile_pool(name="pcr", bufs=1, space="PSUM") as pcr:
                for g in range(NGRP):
                    n0 = G * g
                    x1t, x2t = x1ts[g], x2ts[g]
                    x1f = x1t[:].rearrange("c b k -> c (b k)")
                    x2f = x2t[:].rearrange("c b k -> c (b k)")

                    sq1 = csq.tile([C, G * HW], dt.bfloat16, tag="sq1")
                    nc.gpsimd.tensor_tensor(sq1[:], x1f, x1f, ALU.mult)
                    sq2 = csq.tile([C, G * HW], dt.bfloat16, tag="sq2")
                    nc.scalar.activation(sq2[:], x2f, AF.Square)

                    # compact 4-strip ssq + rsqrt + row-tiled broadcast
                    pn = pnorm.tile([128, 1152], dt.float32, tag="pn")
                    FQ = 288
                    for q in range(4):
                        sq = sq1 if q < 2 else sq2
                        cst = c4 if q < 2 else c16
                        base = 576 * (q % 2)
                        for h in range(2):
                            nc.tensor.matmul(
                                pn[32 * q:32 * q + 32,
                                   512 * h:512 * h + FQ],
                                cst[:], sq[:, base + FQ * h:base + FQ * (h + 1)],
                                start=True, stop=True,
                                tile_position=(0, 32 * q))
                    rcpf = crr.tile([128, 576], dt.float32, tag="rcpf")
                    nc.vector.reciprocal_approx_fast(rcpf[:, 0:288],
                                                     pn[:, 0:288])
                    nc.vector.reciprocal_approx_fast(rcpf[:, 288:576],
                                                     pn[:, 512:800])
                    rcb = crr.tile([128, 576], dt.bfloat16, tag="rcb")
                    nc.scalar.activation(rcb[:], rcpf[:], AF.Sqrt)
                    if dbg and g == 0:
                        nc.sync.dma_start(drcb_d[:, :], rcb[:])

                    x1n = cxn.tile([C, G * HW], dt.bfloat16, tag="x1n")
                    x2n = cxn.tile([C, G * HW], dt.bfloat16, tag="x2n")
                    # r1 broadcast (strips 0,1) -> pn, x1n; then r2 reuses pn
                    for (xn, xf, q0) in ((x1n, x1f, 0), (x2n, x2f, 2)):
                        for (lo, hi) in ((0, 512), (512, 576),
                                         (576, 1024), (1024, 1152)):
                            q = q0 + lo // 576
                            ql = 576 * (lo // 576)
                            nc.tensor.matmul(
                                pn[:, lo:hi],
                                ones128[32 * q:32 * q + 32, :],
                                rcb[32 * q:32 * q + 32,
                                    lo - ql:hi - ql],
                                start=True, stop=True,
                                tile_position=(32 * q, 0))
                        nc.vector.tensor_tensor(xn[:], xf, pn[:, 0:1152],
                                                ALU.mult)

                    # 3 batches per 512-fp32 psum bank: no matmul output
                    # straddles or shares offsets within a bank
                    ca = pca.tile([128, 3, 512], dt.float32, tag="ca")
                    car = pcr.tile([128, 128], dt.float32, tag="car")
                    ccr = pcr.tile([16, 128], dt.float32, tag="ccr")
                    for b in range(G):
                        f0 = HW * b
                        nc.tensor.matmul(ca[:, b // 3,
                                            HW * (b % 3):HW * (b % 3) + HW],
                                         x1n[:, f0:f0 + 128],
                                         x2n[:, f0:f0 + HW],
                                         start=True, stop=True)
                        # swapped residue: out[k<128, ijr]
                        nc.tensor.matmul(car[:, 16 * b:16 * b + 16],
                                         x2n[:, f0:f0 + 128],
                                         x1n[:, f0 + 128:f0 + HW],
                                         start=True, stop=True)
                        # corner: out[kr, ijr]
                        nc.tensor.matmul(ccr[:, 16 * b:16 * b + 16],
                                         x2n[:, f0 + 128:f0 + HW],
                                         x1n[:, f0 + 128:f0 + HW],
                                         start=True, stop=True)
                    # drains (fp8): natural (b, k) order on both sides
                    nc.scalar.activation(
                        XM[:, n0:n0 + 6, 0:HW].rearrange(
                            "p (B s) k -> p B s k", B=2),
                        ca[:, 0:2, 0:3 * HW].rearrange(
                            "p B (s k) -> p B s k", s=3),
                        AF.Copy)
                    nc.scalar.activation(
                        XM[:, n0 + 6:n0 + 8, 0:HW],
                        ca[:, 2, 0:2 * HW].rearrange("p (s k) -> p s k", s=2),
                        AF.Copy)
                    nc.vector.tensor_copy(
                        XM[:, n0:n0 + G, HW:160],
                        car[:].rearrange("p (b r) -> p b r", b=G))
                    nc.vector.tensor_copy(
                        X_co[:, n0:n0 + G, :].rearrange("p b r -> p (b r)"),
                        ccr[:])

            if dbg:
                nc.sync.dma_start(
                    dXa_d[:].rearrange("p (n k) -> p n k", n=N),
                    XM[:, :, 0:HW])
                nc.sync.dma_start(
                    dXsw_d[:].rearrange("p (n r) -> p n r", n=N),
                    XM[:, :, HW:160])
            # corner regroup: X_cp[16*il+kr, j, n] = X_co[kr, n, 8j+il]
            for il in range(8):
                for j in range(2):
                    nc.sync.dma_start(X_cp[16 * il:16 * il + 16, j, :],
                                      X_co[:, :, 8 * j + il])

            sync2_out = dramp.tile([NCORES, 64], dt.bfloat16,
                                   addr_space="Shared")
            nc.gpsimd.collective_compute(
                "AllGather", mybir.AluOpType.bypass, replica_groups=rg,
                ins=[sync_in[:]], outs=[sync2_out[:]])
            if dbg:
                nc.sync.dma_start(dXcp_d[:, :],
                                  X_cp[:].rearrange("p j n -> p (j n)"))
            # small weights for L3/L4 — load during L1
            w3sb = persist.tile([128, 11, D3], dt.bfloat16)
            nc.scalar.dma_start(w3sb[:], w3f_d[:].rearrange("(t p) c -> p t c", p=128))
            b3row = persist.tile([1, D3], dt.bfloat16)
            nc.scalar.dma_start(b3row[:], b3_d[:, :])
            w4sb = persist.tile([128, 3, 8], dt.bfloat16)
            nc.scalar.dma_start(w4sb[:], w4p_d[:].rearrange("(t p) c -> p t c", p=128))
            b4row = persist.tile([1, 8], dt.bfloat16)
            nc.scalar.dma_start(b4row[:], b4_d[:, :])
            b2row = persist.tile([1, D2], dt.bfloat16)
            nc.scalar.dma_start(b2row[:], b2f_d[:, :])

            # ---------------- L1: 81 DoubleRow pair matmuls ----------------
            h1sb = persist.tile([64, COLS1], dt.bfloat16)
            w1v = w1sb[:].rearrange("p (t j c) -> p t j c", t=NPAIR, j=2)
            with tc.tile_pool(name="bias", bufs=1) as biasp, \
                 tc.tile_pool(name="ph1", bufs=1, space="PSUM") as ph1:
                b1row = biasp.tile([1, COLS1], dt.bfloat16, tag="b1")
                nc.scalar.dma_start(b1row[:], b1s_d[:, :])
                h1ps = [ph1.tile([64, 324], dt.float32, tag=f"h1ps{h}",
                                 name=f"h1ps{h}") for h in range(2)]
                XMv = XM[:].rearrange("p n (m j i) -> p m i j n",
                                      m=5, j=2, i=16)
                for t in range(NPAIR):
                    if t < 80:
                        lhsT = XMv[:, t // 16, t % 16, :, :]
                    else:
                        lhsT = X_cp[:, :, :]
                    for h in range(2):
                        nc.tensor.matmul(h1ps[h][:], lhsT,
                                         w1v[:, t, :, 324 * h:324 * h + 324],
                                         start=(t == 0), stop=False,
                                         perf_mode=DR)
                for h in range(2):
                    nc.tensor.matmul(h1ps[h][:], onesrow[:],
                                     b1row[:, 324 * h:324 * h + 324],
                                     start=False, stop=True,
                                     skip_group_check=True)
                for h in range(2):
                    nc.scalar.activation(h1sb[:, 324 * h:324 * h + 324],
                                         h1ps[h][:], AF.Relu,
                                         scale=1.0 / 512.0)

            if dbg:
                nc.sync.dma_start(dh1_d[:, :], h1sb[:])
            # transpose h1 -> fp8 pairs [128, 6, 64] (649..768 zero)
            h1T = persist.tile([128, 6, N], dt.float8e4)
            nc.vector.memset(h1T[:], 0.0)
            with tc.tile_pool(name="ptp", bufs=2, space="PSUM") as ptp:
                for u in range(6):
                    w = 128 if u < 5 else COLS1 - 5 * 128  # 8
                    tp = ptp.tile([128, 64], dt.bfloat16, tag="tp")
                    nc.tensor.transpose(tp[0:w, :], h1sb[:, 128 * u:128 * u + w],
                                        ident[0:64, 0:64])
                    nc.vector.tensor_copy(h1T[0:w, u, :], tp[0:w, :])

            # ---------------- L2 partial (DoubleRow) + AllToAll ------------
            w2v = w2sb[:].rearrange("p (t c) -> p t c", t=6)
            with tc.tile_pool(name="l2", bufs=1) as l2p, \
                 tc.tile_pool(name="ph2", bufs=1, space="PSUM") as ph2:
                h2ps = [ph2.tile([64, 432], dt.float32, tag=f"h2ps{h}",
                                 name=f"h2ps{h}") for h in range(3)]
                for u in range(3):
                    for h in range(3):
                        nc.tensor.matmul(
                            h2ps[h][:], h1T[:, 2 * u:2 * u + 2, :],
                            w2v[:, 2 * u:2 * u + 2, 432 * h:432 * h + 432],
                            start=(u == 0), stop=False, perf_mode=DR)
                for h in range(3):
                    nc.tensor.matmul(h2ps[h][:], onesrow[:],
                                     b2row[:, 432 * h:432 * h + 432],
                                     start=False, stop=True,
                                     skip_group_check=True)
                h2bf = l2p.tile([64, D2], dt.bfloat16, tag="h2bf")
                for h in range(3):
                    nc.scalar.activation(h2bf[:, 432 * h:432 * h + 432],
                                         h2ps[h][:], AF.Copy,
                                         scale=1.0 / 262144.0)
                if dbg:
                    nc.sync.dma_start(dh2_d[:, :], h2bf[:])
                nc.sync.dma_start(a2a_in[:, :], h2bf[:])
                nc.gpsimd.collective_compute(
                    "AllToAll", mybir.AluOpType.bypass, replica_groups=rg,
                    ins=[a2a_in[:]], outs=[a2a_out[:]])

            # ---------------- L3 on this core's 8 batches ----------------
            h2T = persist.tile([128, 11, NB], dt.bfloat16)
            nc.vector.memset(h2T[:], 0.0)
            h3sb = persist.tile([NB, D3], dt.bfloat16)
            with tc.tile_pool(name="l3", bufs=1) as l3p, \
                 tc.tile_pool(name="ptp2", bufs=2, space="PSUM") as ptp2, \
                 tc.tile_pool(name="ph3s", bufs=1, space="PSUM") as ph3s, \
                 tc.tile_pool(name="ph3", bufs=1, space="PSUM") as ph3:
                a2sb = l3p.tile([N, D2], dt.bfloat16, tag="a2sb")
                nc.sync.dma_start(a2sb[:], a2a_out[:, :])
                h2r = l3p.tile([NB, D2], dt.bfloat16, tag="h2r")
                for h in range(3):
                    hp = ph3s.tile([NB, 432], dt.float32, tag=f"h2s{h}",
                                   name=f"h2s{h}")
                    nc.tensor.matmul(hp[:], selT[:],
                                     a2sb[:, 432 * h:432 * h + 432],
                                     start=True, stop=True)
                    nc.scalar.activation(h2r[:, 432 * h:432 * h + 432],
                                         hp[:], AF.Relu)
                for t in range(11):
                    w = 128 if t < 10 else D2 - 10 * 128  # 16
                    tp = ptp2.tile([128, NB], dt.bfloat16, tag="tp2")
                    nc.tensor.transpose(tp[0:w, :], h2r[:, 128 * t:128 * t + w],
                                        ident[0:NB, 0:NB])
                    nc.vector.tensor_copy(h2T[0:w, t, :], tp[0:w, :])
                h3ps = ph3.tile([NB, D3], dt.float32, tag="h3ps")
                for t in range(11):
                    nc.tensor.matmul(h3ps[:], h2T[:, t, :], w3sb[:, t, :],
                                     start=(t == 0), stop=False)
                nc.tensor.matmul(h3ps[:], onesrow[:, 0:NB], b3row[:],
                                 start=False, stop=True)
                nc.scalar.activation(h3sb[:], h3ps[:], AF.Tanh)

            # ---------------- L4 ----------------
            h3T = persist.tile([128, 3, NB], dt.bfloat16)
            nc.vector.memset(h3T[:], 0.0)
            with tc.tile_pool(name="ptp3", bufs=2, space="PSUM") as ptp3, \
                 tc.tile_pool(name="l4", bufs=1) as l4p, \
                 tc.tile_pool(name="ph4", bufs=1, space="PSUM") as ph4:
                for t in range(3):
                    w = 128 if t < 2 else D3 - 256  # 68
                    tp = ptp3.tile([128, NB], dt.bfloat16, tag="tp3")
                    nc.tensor.transpose(tp[0:w, :], h3sb[:, 128 * t:128 * t + w],
                                        ident[0:NB, 0:NB])
                    nc.vector.tensor_copy(h3T[0:w, t, :], tp[0:w, :])
                outps = ph4.tile([NB, 8], dt.float32, tag="outps")
                for t in range(3):
                    nc.tensor.matmul(outps[:], h3T[:, t, :], w4sb[:, t, :],
                                     start=(t == 0), stop=False)
                nc.tensor.matmul(outps[:], onesrow[:, 0:NB], b4row[:],
                                 start=False, stop=True)
                outsb = l4p.tile([NB, 8], dt.float32, tag="outsb")
                nc.vector.tensor_copy(outsb[:], outps[:])
                nc.sync.dma_start(out_d[:, :], outsb[:])

    nc.compile()
    return nc


def _build_w1m(w1core):
    """w1core: [20736, 648] fp32 (already * 1024).
    Returns [128, NPAIR*2*PADC] fp8 in the pair-tile layout."""
    out = np.zeros((128, NPAIR * 2 * PADC), dtype=ml_dtypes.float8_e4m3)
    q = _fp8(w1core)
    kt = q.reshape(HW, HW, COLS1)        # [k, ij, col]
    # pairs t<80: kappa = 32*(t//16) + 16*j + t%16
    #   kappa < 144: main tile  kk = kappa*144 + p      (p = ij 0:128)
    #   kappa >=144: swres tile kk = p*144 + 128 + (kappa-144)  (p = k)
    for t in range(80):
        m, i = t // 16, t % 16
        for j in range(2):
            kappa = 32 * m + 16 * j + i
            base = t * 2 * PADC + j * PADC
            if kappa < HW:
                out[:, base:base + COLS1] = kt[kappa, 0:128, :]
            else:
                out[:, base:base + COLS1] = kt[0:128, 128 + (kappa - HW), :]
    # corner t=80: p = 16*il + kr ; kk = (128+kr)*144 + 128 + (8j+il)
    t = 80
    for j in range(2):
        base = t * 2 * PADC + j * PADC
        for il in range(8):
            for kr in range(16):
                out[16 * il + kr, base:base + COLS1] = \
                    kt[128 + kr, 128 + 8 * j + il, :]
    return out


def _prep_inputs(x1, x2, w1, b1, w2, b2, w3, b3, w4, b4):
    x1f = np.asarray(x1, np.float32).reshape(N, C, HW)
    x2f = np.asarray(x2, np.float32).reshape(N, C, HW)
    x1q = _fp8(np.ascontiguousarray(x1f.transpose(1, 0, 2)))
    x2q = _fp8(np.ascontiguousarray(x2f.transpose(1, 0, 2)))
    w1 = np.asarray(w1, np.float32)
    w2 = np.asarray(w2, np.float32)
    w3 = np.asarray(w3, np.float32)
    w4 = np.asarray(w4, np.float32)
    b1 = np.asarray(b1, np.float32)
    b2 = np.asarray(b2, np.float32)

    w3pad = np.zeros((PADK3, D3), np.float32)
    w3pad[:D2] = w3
    w4pad = np.zeros((PADK4, 8), np.float32)
    w4pad[:D3] = w4
    w3b = _bf16(w3pad)
    w4b = _bf16(w4pad)
    b2f = _bf16(b2 / NCORES * 262144.0).reshape(1, D2)
    b3r = _bf16(b3).reshape(1, D3)
    b4r = _bf16(b4).reshape(1, 8)

    in_maps = []
    for core in range(NCORES):
        w1c = w1[:, COLS1 * core:COLS1 * (core + 1)] * 1024.0
        w1m = _build_w1m(w1c)
        # w2 rows for this core's h1 slice, *1024, padded to 768, [p, t*c]
        w2c = np.zeros((768, D2), np.float32)
        w2c[:COLS1] = w2[COLS1 * core:COLS1 * (core + 1)] * 1024.0
        w2m = _fp8(np.ascontiguousarray(
            w2c.reshape(6, 128, D2).transpose(1, 0, 2).reshape(128, 6 * D2)))
        in_maps.append({
            "x1q": x1q, "x2q": x2q,
            "w1m": w1m,
            "b1s": _bf16(b1[COLS1 * core:COLS1 * (core + 1)]
                         * 131072.0).reshape(1, COLS1),
            "w2m": w2m,
            "b2f": b2f,
            "w3f": w3b,
            "b3r": b3r,
            "w4p": w4b,
            "b4r": b4r,
        })
    return in_maps


def kernel(x1, x2, w1, b1, w2, b2, w3, b3, w4, b4):
    global LAST_RESULT
    from concourse.bass_utils import run_bass_kernel_spmd

    if "nc" not in _CACHE:
        _CACHE["nc"] = _build_nc()
    nc = _CACHE["nc"]

    in_maps = _prep_inputs(x1, x2, w1, b1, w2, b2, w3, b3, w4, b4)
    trace = bool(int(os.environ.get("HNET_TRACE", "0")))
    res = run_bass_kernel_spmd(nc, in_maps, core_ids=list(range(NCORES)),
                               trace=trace)
    LAST_RESULT = res
    H = np.concatenate(
        [np.asarray(res.results[c]["out"], np.float32) for c in range(NCORES)],
        axis=0)
    ones = np.ones((N, 1), np.float32)
    return np.concatenate([H, ones], axis=1).reshape(N, 3, 3)
